# revision 55
# baseline (speedup 1.0000x reference)
"""Trainium2 8-core tensor-parallel attention kernel (Bass/Tile).

Full inputs in, full output out. Tensor-parallel over heads (4 per core).
Fused schedule: proj(b0) -> attn(b0) || proj(b1) -> attn(b1) || o_proj,
with one AllGather per (batch, 512-query chunk) overlapped with compute,
and o_proj chunks emission-interleaved with attention.
Causal structure exploited at 128-column granularity on the diagonal.
All hot DMA paths are fully contiguous (slice-major x on the host,
per-(batch,head,chunk) q/k/v DRAM tiles, per-feature-block o_proj loads).
RoPE rotation done by a partition pair-swap DMA with the sign folded
into sin (no tensor-engine rotation matmuls).
"""
import sys

for _p in ("/opt/trn_rl_repo",):
    if _p not in sys.path:
        sys.path.insert(0, _p)

import numpy as np
import ml_dtypes

import concourse.bass as bass
import concourse.mybir as mybir
import concourse.tile as tile
from concourse import bacc
from concourse.bass_utils import run_bass_kernel_spmd

B, S, D, H = 2, 2048, 4096, 32
HD = D // H          # 128 head dim
T = B * S            # 4096 tokens
NC = 8               # cores
HL = H // NC         # 4 heads per core
DH = HL * HD         # 512 dims per core
SCALE = 1.0 / float(np.sqrt(HD))
BF16 = mybir.dt.bfloat16
F32 = mybir.dt.float32
bf16 = ml_dtypes.bfloat16

NT = T // 512        # 8 token slices of 512
NSB = S // 512       # 4 slices per batch
NCT = D // 128       # 32 contraction tiles
NKB = S // 128       # 16 key blocks per batch

_CACHE = {}
LAST_RESULT = None


def build():
    nc = bacc.Bacc("TRN2", target_bir_lowering=False, debug=False, num_devices=NC)

    # x stored slice-major with contraction blocks paired side by side:
    # 2KB SBUF partition lines on every x tile load
    xS = nc.dram_tensor("xS", [NT, D // 2, 1024], BF16, kind="ExternalInput").ap()
    # qkv weights pre-paired on the host: block p holds contraction blocks
    # 2p and 2p+1 side by side -> 2KB SBUF partition lines on the load
    wqT = nc.dram_tensor("wqT", [D // 2, 2 * DH], BF16, kind="ExternalInput").ap()
    wkT = nc.dram_tensor("wkT", [D // 2, 2 * DH], BF16, kind="ExternalInput").ap()
    wvT = nc.dram_tensor("wvT", [D // 2, 2 * DH], BF16, kind="ExternalInput").ap()
    woT = nc.dram_tensor("woT", [D, DH], BF16, kind="ExternalInput").ap()
    cosE = nc.dram_tensor("cosE", [HD, S], BF16, kind="ExternalInput").ap()
    sinE = nc.dram_tensor("sinE", [HD, S], BF16, kind="ExternalInput").ap()  # sign-folded
    mask128 = nc.dram_tensor("mask128", [128, 128], F32, kind="ExternalInput").ap()
    ones128 = nc.dram_tensor("ones128", [128, 1], BF16, kind="ExternalInput").ap()
    out = nc.dram_tensor("out", [T, DH], F32, kind="ExternalOutput").ap()

    with tile.TileContext(nc) as tc:
        with tc.tile_pool(name="dram", bufs=1, space="DRAM") as dram, \
             tc.tile_pool(name="cons", bufs=1) as cons, \
             tc.tile_pool(name="qkh", bufs=1) as qkh, \
             tc.tile_pool(name="vh", bufs=5) as vhp, \
             tc.tile_pool(name="aw", bufs=1) as aw, \
             tc.tile_pool(name="aps", bufs=1, space="PSUM") as aps:

            # ---- DRAM internals: per (b, head, jq-chunk), all contiguous ----
            qd, kd, vd = {}, {}, {}
            for b in range(B):
                for h in range(HL):
                    for jq in range(NSB):
                        qd[(b, h, jq)] = dram.tile([128, 512], BF16, name=f"qd{b}{h}{jq}")
                        kd[(b, h, jq)] = dram.tile([128, 512], BF16, name=f"kd{b}{h}{jq}")
                        vd[(b, h, jq)] = dram.tile([128, 512], BF16, name=f"vd{b}{h}{jq}")
            agin, agout = {}, {}
            for b in range(B):
                for jq in range(NSB):
                    agin[(b, jq)] = dram.tile([DH, 512], BF16, name=f"agin{b}{jq}")
                    agout[(b, jq)] = dram.tile([NC * DH, 512], BF16,
                                               addr_space="Shared", name=f"agout{b}{jq}")
            # per-head split for the last two chunks (b=1, jq in {3, 0}): the
            # gather starts after each head instead of after the whole chunk
            agin_h, agout_h = {}, {}
            for jq in (3, 0):
                for h in range(HL):
                    agin_h[(jq, h)] = dram.tile([128, 512], BF16, name=f"aginh{jq}{h}")
                    agout_h[(jq, h)] = dram.tile([NC * 128, 512], BF16,
                                                 addr_space="Shared", name=f"agouth{jq}{h}")

            # ---- constants ----
            mask_sb = cons.tile([128, 128], F32, name="mask_sb")
            nc.sync.dma_start(mask_sb[:], mask128[:])
            o128_sb = cons.tile([128, 1], BF16, name="o128_sb")
            nc.sync.dma_start(o128_sb[:], ones128[:])

            def load_x(px, t):
                xt = []
                for p in range(NCT // 2):
                    xp = px.tile([128, 1024], BF16, tag="xt", bufs=16, name=f"x{t}_{p}")
                    nc.sync.dma_start(xp[:], xS[t, p * 128:(p + 1) * 128, :])
                    for u in range(2):
                        xt.append(xp[:, u * 512:(u + 1) * 512])
                return xt

            def proj_slice(pw, px, prw, pps, t, xt_first=None, split_first=False):
                    b = t // NSB
                    jq = t % NSB
                    off = jq * 512
                    cos_t = prw.tile([128, 512], BF16, tag="cos", name=f"cos{t}")
                    sin_t = prw.tile([128, 512], BF16, tag="sin", name=f"sin{t}")
                    nc.sync.dma_start(cos_t[:], cosE[:, off:off + 512])
                    nc.sync.dma_start(sin_t[:], sinE[:, off:off + 512])
                    xt = xt_first if xt_first is not None else load_x(px, t)

                    def rope_drain(wname, i, ps):
                            dst = qd if wname == "q" else kd
                            qsb = prw.tile([128, 512], BF16, tag="qsb", name=f"qq{wname}{t}{i}")
                            nc.any.tensor_copy(qsb[:], ps[:])
                            # pair-swap via SBUF->SBUF DMA (sign folded into sinE)
                            qsw = prw.tile([128, 512], BF16, tag="qsw", name=f"qw{wname}{t}{i}")
                            qsb_r = qsb.rearrange("(d two) n -> two d n", two=2)
                            qsw_r = qsw.rearrange("(d two) n -> two d n", two=2)
                            nc.sync.dma_start(qsw_r[0], qsb_r[1])
                            nc.sync.dma_start(qsw_r[1], qsb_r[0])
                            qc = prw.tile([128, 512], BF16, tag="qc", name=f"qc{wname}{t}{i}")
                            nc.vector.tensor_tensor(qc[:], ps[:], cos_t[:], mybir.AluOpType.mult)
                            qr = prw.tile([128, 512], BF16, tag="qr", name=f"qr{wname}{t}{i}")
                            nc.vector.tensor_tensor(qr[:], qsw[:], sin_t[:], mybir.AluOpType.mult)
                            qf = prw.tile([128, 512], BF16, tag="qf", name=f"qf{wname}{t}{i}")
                            nc.vector.tensor_tensor(qf[:], qc[:], qr[:], mybir.AluOpType.add)
                            nc.sync.dma_start(dst[(b, i, jq)][:], qf[:])

                    if split_first:
                        # slice 0: pair up q psum groups, half contraction each,
                        # so the PE starts before the full 8MB of x+wq lands
                        ps_open = {}
                        for i, ha in [(0, 0), (1, 0), (0, 1), (2, 0), (1, 1),
                                      (3, 0), (2, 1), (3, 1)]:
                            if ha == 0:
                                ps_open[i] = pps.tile([128, 512], F32, tag="pp", bufs=2,
                                                      name=f"psq{t}{i}")
                            ps = ps_open[i]
                            for c in range(16 * ha, 16 * ha + 16):
                                nc.tensor.matmul(
                                    ps[:], pw[("q", c)][:, i * 128:(i + 1) * 128],
                                    xt[c][:], start=(c == 0), stop=(c == NCT - 1))
                            if ha == 1:
                                rope_drain("q", i, ps)
                        qk_groups = [("k", i) for i in range(HL)]
                    else:
                        qk_groups = [("q", i) for i in range(HL)] + [("k", i) for i in range(HL)]

                    for wname, i in qk_groups:
                        ps = pps.tile([128, 512], F32, tag="pp", bufs=2, name=f"ps{wname}{t}{i}")
                        for c in range(NCT):
                            nc.tensor.matmul(
                                ps[:], pw[(wname, c)][:, i * 128:(i + 1) * 128],
                                xt[c][:], start=(c == 0), stop=(c == NCT - 1))
                        rope_drain(wname, i, ps)
                    # v projection (token-major), tt-pairs, wv resident
                    for half in range(2):
                        psv = []
                        for u in range(2):
                            p = pps.tile([128, 512], F32, tag="pp", bufs=2, name=f"psv{t}{half}{u}")
                            psv.append(p)
                        for c in range(NCT):
                            for u in range(2):
                                tt = half * 2 + u
                                nc.tensor.matmul(
                                    psv[u][:], xt[c][:, tt * 128:(tt + 1) * 128],
                                    pw[("v", c)][:], start=(c == 0), stop=(c == NCT - 1))
                        for u in range(2):
                            tt = half * 2 + u
                            vst = prw.tile([128, 512], BF16, tag="vst", bufs=2, name=f"vs{t}{half}{u}")
                            nc.any.tensor_copy(vst[:], psv[u][:])
                            for h in range(HL):
                                nc.sync.dma_start(
                                    vd[(b, h, jq)][:, tt * 128:(tt + 1) * 128],
                                    vst[:, h * 128:(h + 1) * 128])

            attn_state = {}

            def attn_setup_alloc(b):
                vh, qh, kh = {}, {}, {}
                for h in range(HL):
                    qh[h] = qkh.tile([128, S], BF16, tag="qh", bufs=4, name=f"qh{b}{h}")
                    kh[h] = qkh.tile([128, S], BF16, tag="kh", bufs=4, name=f"kh{b}{h}")
                    vh[h] = vhp.tile([128, NKB * 128], BF16, tag="vh", name=f"vh{b}{h}")
                attn_state[b] = (qh, kh, vh)

            def attn_load_chunk(b, jq):
                # MUST be emitted after proj slice (b, jq) so the DRAM reads
                # order after the writes
                qh, kh, vh = attn_state[b]
                sl = slice(jq * 512, (jq + 1) * 512)
                for h in range(HL):
                    nc.sync.dma_start(qh[h][:, sl], qd[(b, h, jq)][:])
                    nc.sync.dma_start(kh[h][:, sl], kd[(b, h, jq)][:])
                    nc.sync.dma_start(vh[h][:, sl], vd[(b, h, jq)][:])

            def attn_chunk(b, jq, per_head_ag=False):
                qh, kh, vh = attn_state[b]
                for h in range(HL):
                    acc = aps.tile([128, 512], F32, tag="acc", bufs=2, name=f"acc{b}{jq}{h}")
                    # exp tiles accumulate on DVE; one sums matmul per block
                    exa = aw.tile([128, 512], BF16, tag="exa", bufs=2, name=f"exa{b}{jq}{h}")
                    nkt = 4 * (jq + 1)
                    for kt in range(nkt):
                        diag = kt >= 4 * jq
                        m = kt - 4 * jq
                        qoff = jq * 512 + (m * 128 if diag else 0)
                        n = 512 - (m * 128 if diag else 0)
                        ro = qoff - jq * 512      # offset within acc/exa
                        pss = aps.tile([128, 512], F32, tag="pss", bufs=3, name=f"pss{b}{jq}{h}{kt}")
                        nc.tensor.matmul(
                            pss[:, :n], kh[h][:, kt * 128:(kt + 1) * 128],
                            qh[h][:, qoff:jq * 512 + 512], start=True, stop=True)
                        if diag:
                            nc.vector.tensor_tensor(
                                pss[:, :128], pss[:, :128], mask_sb[:],
                                mybir.AluOpType.add)
                        if kt == 0:
                            ex = exa           # exp writes the accumulator directly
                        else:
                            ex = aw.tile([128, 512], BF16, tag="ex", bufs=4, name=f"ex{b}{jq}{h}{kt}")
                        nc.scalar.activation(ex[:, :n], pss[:, :n],
                                             mybir.ActivationFunctionType.Exp, scale=SCALE)
                        nc.tensor.matmul(acc[:, ro:512], vh[h][:, kt * 128:(kt + 1) * 128],
                                         ex[:, :n], start=(kt == 0), stop=(kt == nkt - 1))
                        if kt > 0:
                            nc.vector.tensor_tensor(exa[:, ro:512], exa[:, ro:512],
                                                    ex[:, :n], mybir.AluOpType.add)
                    sums = aps.tile([1, 512], F32, tag="sums", bufs=1, name=f"sums{b}{jq}{h}")
                    nc.tensor.matmul(sums[:], o128_sb[:], exa[:], start=True, stop=True)
                    rec = aw.tile([1, 512], BF16, tag="rec", bufs=1, name=f"rec{b}{jq}{h}")
                    with nc.allow_low_precision(reason="softmax denom reciprocal in bf16 is fine at 2e-2 tol"):
                        nc.vector.reciprocal(rec[:], sums[:])
                    rbs = aw.tile([128, 512], BF16, tag="rbs", bufs=2, name=f"rbs{b}{jq}{h}")
                    nc.gpsimd.partition_broadcast(rbs[:], rec[:])
                    att = aw.tile([128, 512], BF16, tag="att", bufs=2, name=f"att{b}{jq}{h}")
                    nc.vector.tensor_tensor(att[:], acc[:], rbs[:], mybir.AluOpType.mult)
                    if per_head_ag:
                        nc.sync.dma_start(agin_h[(jq, h)][:], att[:])
                        nc.gpsimd.collective_compute(
                            "AllGather", mybir.AluOpType.bypass,
                            replica_groups=[list(range(NC))],
                            ins=[agin_h[(jq, h)].opt()], outs=[agout_h[(jq, h)].opt()])
                    else:
                        nc.sync.dma_start(agin[(b, jq)][h * 128:(h + 1) * 128, :], att[:])
                if not per_head_ag:
                    nc.gpsimd.collective_compute(
                        "AllGather", mybir.AluOpType.bypass,
                        replica_groups=[list(range(NC))],
                        ins=[agin[(b, jq)].opt()], outs=[agout[(b, jq)].opt()])

            # ================= emission =================
            with tc.tile_pool(name="pw", bufs=1) as pwp, \
                 tc.tile_pool(name="px", bufs=1) as px, \
                 tc.tile_pool(name="prw", bufs=2) as prw, \
                 tc.tile_pool(name="pps", bufs=1, space="PSUM") as pps:
                # interleave x slice 0 with wq: first psum group's deps land
                # first; weights come as 2-block pairs (2KB partition lines)
                pw = {}
                xt0 = []
                for p in range(NCT // 2):
                    xp = px.tile([128, 1024], BF16, tag="xt", bufs=16, name=f"x0_{p}")
                    nc.sync.dma_start(xp[:], xS[0, p * 128:(p + 1) * 128, :])
                    for u in range(2):
                        xt0.append(xp[:, u * 512:(u + 1) * 512])
                    wt = pwp.tile([128, 2 * DH], BF16, name=f"wq_{p}")
                    nc.sync.dma_start(wt[:], wqT[p * 128:(p + 1) * 128, :])
                    for u in range(2):
                        pw[("q", 2 * p + u)] = wt[:, u * DH:(u + 1) * DH]
                for wname, w_dr in (("k", wkT), ("v", wvT)):
                    for p in range(NCT // 2):
                        wt = pwp.tile([128, 2 * DH], BF16, name=f"w{wname}_{p}")
                        nc.sync.dma_start(wt[:], w_dr[p * 128:(p + 1) * 128, :])
                        for u in range(2):
                            pw[(wname, 2 * p + u)] = wt[:, u * DH:(u + 1) * DH]
                # attention chunks interleave into projection emission as soon
                # as their dependency slices are written -> earlier gathers
                proj_slice(pw, px, prw, pps, 0, xt_first=xt0, split_first=True)
                attn_setup_alloc(0)
                attn_load_chunk(0, 0)
                proj_slice(pw, px, prw, pps, 1)
                attn_load_chunk(0, 1)
                attn_chunk(0, 0)
                proj_slice(pw, px, prw, pps, 2)
                attn_load_chunk(0, 2)
                attn_chunk(0, 1)
                proj_slice(pw, px, prw, pps, 3)
                attn_load_chunk(0, 3)
                attn_chunk(0, 2)
                proj_slice(pw, px, prw, pps, 4)
                attn_setup_alloc(1)
                attn_load_chunk(1, 0)
                attn_chunk(0, 3)
                proj_slice(pw, px, prw, pps, 5)
                attn_load_chunk(1, 1)
                attn_chunk(1, 1)
                proj_slice(pw, px, prw, pps, 6)
                attn_load_chunk(1, 2)
                attn_chunk(1, 2)
                proj_slice(pw, px, prw, pps, 7)
                attn_load_chunk(1, 3)
                attn_chunk(1, 3, per_head_ag=True)
                attn_chunk(1, 0, per_head_ag=True)
            # projection pools (weights, x, rope work, proj psum) released here

            with tc.tile_pool(name="ores", bufs=1) as ores, \
                 tc.tile_pool(name="och", bufs=1) as och, \
                 tc.tile_pool(name="oo", bufs=6) as oo, \
                 tc.tile_pool(name="ops", bufs=2, space="PSUM") as ops:
                # first chunk's gathers issued before the (big) wo load so the
                # two DMA streams run in parallel
                ch00 = []
                for c in range(NCT):
                    cc = och.tile([128, 512], BF16, tag="ch", bufs=96, name=f"ch00_{c}")
                    nc.sync.dma_start(cc[:], agout[(0, 0)][c * 128:(c + 1) * 128, :])
                    ch00.append(cc)
                wo_sb = ores.tile([128, NCT * DH], BF16, name="wo_sb")
                nc.sync.dma_start(
                    wo_sb[:].rearrange("p (c i) -> p c i", c=NCT),
                    woT.rearrange("(c p) i -> p c i", p=128))

                def oproj_chunk(b, jq, ch_pre=None):
                    # contiguous per-feature-block loads of the gathered chunk
                    ch = ch_pre
                    if ch is None:
                        ch = []
                        for c in range(NCT):
                            cc = och.tile([128, 512], BF16, tag="ch", bufs=96, name=f"ch{b}{jq}{c}")
                            nc.sync.dma_start(cc[:], agout[(b, jq)][c * 128:(c + 1) * 128, :])
                            ch.append(cc)
                    for tt in range(4):
                        pso = ops.tile([128, 512], F32, tag="pso", name=f"pso{b}{jq}{tt}")
                        for c in range(NCT):
                            nc.tensor.matmul(pso[:], ch[c][:, tt * 128:(tt + 1) * 128],
                                             wo_sb[:, c * DH:(c + 1) * DH],
                                             start=(c == 0), stop=(c == NCT - 1))
                        ot = oo.tile([128, 512], F32, tag="ot", name=f"ot{b}{jq}{tt}")
                        nc.any.tensor_copy(ot[:], pso[:])
                        row = b * S + jq * 512 + tt * 128
                        nc.sync.dma_start(out[row:row + 128, :], ot[:])

                def oproj_chunk_h(jq):
                    # b=1 chunk gathered per head: contraction grouped h-major
                    # so early matmuls only need early gathers
                    ch = {}
                    for h in range(HL):
                        for r in range(NC):
                            f = r * HL + h
                            cc = och.tile([128, 512], BF16, tag="ch", bufs=96, name=f"chh{jq}{f}")
                            nc.sync.dma_start(cc[:], agout_h[(jq, h)][r * 128:(r + 1) * 128, :])
                            ch[f] = cc
                    for tt in range(4):
                        pso = ops.tile([128, 512], F32, tag="pso", name=f"psoh{jq}{tt}")
                        for ci, (h, r) in enumerate((h, r) for h in range(HL) for r in range(NC)):
                            f = r * HL + h
                            nc.tensor.matmul(pso[:], ch[f][:, tt * 128:(tt + 1) * 128],
                                             wo_sb[:, f * DH:(f + 1) * DH],
                                             start=(ci == 0), stop=(ci == NCT - 1))
                        ot = oo.tile([128, 512], F32, tag="ot", name=f"oth{jq}{tt}")
                        nc.any.tensor_copy(ot[:], pso[:])
                        row = S + jq * 512 + tt * 128
                        nc.sync.dma_start(out[row:row + 128, :], ot[:])

                oproj_chunk(0, 0, ch_pre=ch00)
                for jq in (1, 2, 3):
                    oproj_chunk(0, jq)
                for jq in (1, 2):
                    oproj_chunk(1, jq)
                oproj_chunk_h(3)
                oproj_chunk_h(0)

    nc.compile()
    return nc


def _host_prep(x, freqs_cos, freqs_sin, mask, wq, wk, wv, wo):
    xT = np.asarray(x, np.float32).reshape(T, D).T        # [D, T]
    xSm = np.empty((NT, D // 2, 2 * 512), np.float32)     # paired blocks
    for t in range(NT):
        blk = xT[:, t * 512:(t + 1) * 512].reshape(NCT, 128, 512)
        xSm[t] = np.concatenate([blk[0::2], blk[1::2]], axis=2).reshape(D // 2, 1024)
    xSm = np.ascontiguousarray(xSm).astype(bf16)
    cos = np.asarray(freqs_cos, np.float32)   # [S, 64]
    sin = np.asarray(freqs_sin, np.float32)
    cosE = np.ascontiguousarray(np.repeat(cos.T, 2, axis=0)).astype(bf16)  # [128, S]
    sinE = np.repeat(sin.T, 2, axis=0)                     # [128, S]
    sinE[0::2, :] *= -1.0                                  # sign fold: even rows negative
    sinE = np.ascontiguousarray(sinE).astype(bf16)
    # causal triangle for a 128x128 diagonal block, pre-scaled for exp(scale*x)
    kk = np.arange(128)
    mask128 = np.where(kk[:, None] > kk[None, :], -1e9 / SCALE, 0.0).astype(np.float32)
    ones128 = np.ones((128, 1), bf16)
    shared = dict(xS=xSm, cosE=cosE, sinE=sinE, mask128=mask128, ones128=ones128)
    in_maps = []
    for r in range(NC):
        sl = slice(r * DH, (r + 1) * DH)
        m = dict(shared)
        def _pairw(wmat):
            wT = np.asarray(wmat, np.float32)[sl, :].T          # [D, DH]
            blk = wT.reshape(NCT, 128, DH)
            pr = np.concatenate([blk[0::2], blk[1::2]], axis=2)  # [16, 128, 1024]
            return np.ascontiguousarray(pr.reshape(D // 2, 2 * DH)).astype(bf16)
        m["wqT"] = _pairw(wq)
        m["wkT"] = _pairw(wk)
        m["wvT"] = _pairw(wv)
        m["woT"] = np.ascontiguousarray(np.asarray(wo, np.float32)[sl, :].T).astype(bf16)
        in_maps.append(m)
    return in_maps


def kernel(x, freqs_cos, freqs_sin, mask, wq, wk, wv, wo, start_pos):
    global LAST_RESULT
    if "nc" not in _CACHE:
        _CACHE["nc"] = build()
    nc = _CACHE["nc"]
    in_maps = _host_prep(x, freqs_cos, freqs_sin, mask, wq, wk, wv, wo)
    res = run_bass_kernel_spmd(nc, in_maps, core_ids=list(range(NC)))
    LAST_RESULT = res
    parts = [res.results[r]["out"] for r in range(NC)]
    full = np.concatenate(parts, axis=1)      # [T, D]
    return np.ascontiguousarray(full.reshape(B, S, D)).astype(np.float32)
